# revision 13
# baseline (speedup 1.0000x reference)
"""Trainium2 Bass kernel for nn_MAMLAwareGANLoss.

Reference computation (B=1024, Z=256, H=W=128, N=H*W=16384):
    fake   = tanh(noise @ Wg)                      # [B, N]
    d_fake = fake @ Wd                             # [B, 1]
    g_loss = mean(softplus(-d_fake))               # (+ 0.0 * sum(d_real) == 0)
    solvability_loss = mean(per-sample flood-fill penalty of (fake == 1.0) walls)
    cur    = mean(fake == 1.0)
    difficulty_loss  = (cur - current_difficulty)^2
    loss   = g_loss + w_s * solvability_loss + w_d * difficulty_loss

Key structural facts used here:
  * real_mazes enters only through `0.0 * sum(d_real)` == exactly 0.0 -> never loaded.
  * "walls" are cells where float32 tanh(x) rounds to exactly 1.0, which requires
    x >= ~9.01.  We prove on the host (Cauchy-Schwarz over the actual inputs:
    max_b ||noise_b|| * max_n ||Wg[:, n]||) that no |x| can exceed the threshold,
    hence wall count == 0 exactly => solvability_loss == 0.0 and cur == 0.0.
    If the bound ever fails we fall back to an exact host recomputation.
  * Therefore the device only computes d_fake = (tanh(noise @ Wg)) @ Wd.

Device sharding (8 cores): shard the N (=H*W) dimension, 2048 columns/core.
Each core computes, for all 1024 samples, the partial dot product
    dpart[b] = sum_{n in shard} tanh((noise @ Wg)[b, n]) * Wd[n]
The host sums the 8 partials, applies softplus and the scalar tail.

Per-core device program (layout: n on PSUM partitions, b on free axis):
    x[n, b]  = sum_z Wg[z, n] * noiseT[z, b]    (PE, fp32r, K=z on partitions)
    t[n, b]  = tanh(x[n, b])                     (ACT, PSUM->SBUF)
    dpart[b] = sum_n Wd[n] * t[n, b]             (PE, stationary = Wd column)
This keeps the weighted reduction on the tensor engine (contraction over
partitions), which is far faster than DVE fp32 reductions.
"""

import numpy as np

B, Z, H, W = 1024, 256, 128, 128
N = H * W               # 16384
NCORES = 8
NSH = N // NCORES       # 2048 columns of Wg per core
P = 128
NT = NSH // P           # 16 n-tiles per core
NB = B                  # 1024 samples (free axis)
CHUNKS = 8              # Wg DMA chunks per core
CW = NSH // CHUNKS      # 256 columns per chunk
TILES_PER_CHUNK = NT // CHUNKS

# float32 tanh(x) rounds to exactly 1.0 only for x >= ~9.01; stay well below.
WALL_SAFE_BOUND = 8.5

_PROG = None  # cached compiled Bass program


def _build_program():
    import concourse.bass as bass
    import concourse.tile as tile
    from concourse import bacc, mybir

    f32 = mybir.dt.float32
    f32r = mybir.dt.float32r
    Tanh = mybir.ActivationFunctionType.Tanh

    nc = bacc.Bacc(
        "TRN2", target_bir_lowering=False, debug=False, num_devices=NCORES
    )
    # Inputs are host-relaid so every DMA source is one contiguous block:
    #   noise_t : [2(z), 2(h), 128, 512]  = noise.T tiled
    #   wg_shard: [CHUNKS, 2(z), 128, CW] = Wg shard tiled
    noise_d = nc.declare_dram_parameter(
        "noise_t", [2, 2, P, 512], f32r, isOutput=False
    )
    wg_d = nc.declare_dram_parameter(
        "wg_shard", [CHUNKS, 2, P, CW], f32r, isOutput=False
    )
    wd_d = nc.declare_dram_parameter("wd_shard", [P, NT], f32r, isOutput=False)
    out_d = nc.declare_dram_parameter("dpart", [1, NB], f32, isOutput=True)

    with tile.TileContext(nc) as tc:
        with (
            tc.tile_pool(name="const", bufs=1) as cpool,
            tc.tile_pool(name="wg", bufs=1) as wgpool,
            tc.tile_pool(name="t", bufs=3) as tpool,
            tc.tile_pool(name="ps", bufs=3, space="PSUM") as pspool,
            tc.tile_pool(name="dps", bufs=1, space="PSUM") as dpool,
        ):
            d_ps = dpool.tile([1, NB], f32, tag="dacc")

            # --- PE warm-up: HAM-unthrottle the tensor engine during the DMA
            # wait using matmuls on a memset tile (no DMA dependency).
            # Plain fp32 matmuls run at quarter rate, so a handful keeps the
            # PE busy for the ~3.4us HAM window.  Output goes to d_ps, which
            # the real reduction later clears with start=True.
            warm_sb = cpool.tile([P, 512], f32, tag="warm")
            nc.gpsimd.memset(warm_sb[:], 0.0)
            for _ in range(4):
                nc.tensor.matmul(
                    d_ps[:, 0:512],
                    warm_sb[:, 0:1],
                    warm_sb[:],
                    start=True,
                    stop=True,
                    skip_group_check=True,
                )

            # DMA issue spread across three engine queues; the blocks that
            # gate the first matmuls are placed at the head of each queue.
            wg_chunks = [
                wgpool.tile([P, 2, CW], f32r, name=f"wg{ci}", tag=f"wg{ci}")
                for ci in range(CHUNKS)
            ]
            noise_sb = cpool.tile([P, 2, NB], f32r, tag="noise")
            wd_sb = cpool.tile([P, NT], f32r, tag="wd")

            nc.sync.dma_start(out=wg_chunks[0][:, 0:1, :], in_=wg_d[0, 0])
            nc.gpsimd.dma_start(out=wg_chunks[0][:, 1:2, :], in_=wg_d[0, 1])
            nc.scalar.dma_start(out=noise_sb[:, 0:1, 0:512], in_=noise_d[0, 0])
            nc.sync.dma_start(out=noise_sb[:, 1:2, 0:512], in_=noise_d[1, 0])
            nc.scalar.dma_start(out=noise_sb[:, 0:1, 512:1024], in_=noise_d[0, 1])
            nc.gpsimd.dma_start(out=noise_sb[:, 1:2, 512:1024], in_=noise_d[1, 1])

            dma_engines = [nc.sync, nc.gpsimd, nc.scalar]
            _ei = [0]

            def dma(out, in_):
                eng = dma_engines[_ei[0] % len(dma_engines)]
                _ei[0] += 1
                eng.dma_start(out=out, in_=in_)

            for ci in range(1, CHUNKS):
                for z in range(2):
                    dma(wg_chunks[ci][:, z : z + 1, :], wg_d[ci, z])
            dma(wd_sb[:], wd_d[:])

            for i in range(NT):
                ci, sub = divmod(i, TILES_PER_CHUNK)
                wt = wg_chunks[ci]
                ps = pspool.tile([P, NB], f32)
                for z in range(2):
                    wg_blk = wt[:, z : z + 1, sub * P : (sub + 1) * P]
                    for h in range(2):
                        nc.tensor.matmul(
                            ps[:, h * 512 : (h + 1) * 512],
                            wg_blk,
                            noise_sb[:, z : z + 1, h * 512 : (h + 1) * 512],
                            start=(z == 0),
                            stop=(z == 1),
                        )
                t = tpool.tile([P, NB], f32r)
                nc.scalar.activation(t[:], ps[:], Tanh)
                wd_col = wd_sb[:, i : i + 1]
                for h in range(2):
                    nc.tensor.matmul(
                        d_ps[:, h * 512 : (h + 1) * 512],
                        wd_col,
                        t[:, h * 512 : (h + 1) * 512],
                        start=(i == 0),
                        stop=(i == NT - 1),
                        skip_group_check=True,
                    )

            out_sb = cpool.tile([1, NB], f32, tag="out")
            nc.scalar.copy(out_sb[:], d_ps[:])
            nc.sync.dma_start(out=out_d[:], in_=out_sb[:])

    nc.compile()
    return nc


def _get_program():
    global _PROG
    if _PROG is None:
        _PROG = _build_program()
    return _PROG


def _make_in_maps(noise, Wg, Wd):
    # noise.T tiled into contiguous [z, h, 128, 512] blocks
    nt = noise.T  # [Z, B]
    noise_t = np.ascontiguousarray(
        nt.reshape(2, P, 2, 512).transpose(0, 2, 1, 3)
    )  # [2, 2, 128, 512]
    in_maps = []
    for c in range(NCORES):
        wg_c = Wg[:, c * NSH : (c + 1) * NSH]  # [Z, NSH]
        # -> contiguous [chunk, z, 128, CW] blocks
        wg_t = np.ascontiguousarray(
            wg_c.reshape(2, P, CHUNKS, CW).transpose(2, 0, 1, 3)
        )
        seg = Wd[c * NSH : (c + 1) * NSH, 0]
        wd_c = np.ascontiguousarray(seg.reshape(NT, P).T)  # [P, NT]
        in_maps.append({"noise_t": noise_t, "wg_shard": wg_t, "wd_shard": wd_c})
    return in_maps


def run_device(noise, Wg, Wd, trace=False):
    """Run the SPMD kernel on 8 cores; return (d_fake[B] float64, results)."""
    from concourse.bass_utils import run_bass_kernel_spmd

    nc = _get_program()
    in_maps = _make_in_maps(noise, Wg, Wd)
    res = run_bass_kernel_spmd(nc, in_maps, list(range(NCORES)), trace=trace)
    d_fake = np.zeros(NB, np.float64)
    for r in res.results:
        d_fake += np.asarray(r["dpart"], np.float64).reshape(NB)
    return d_fake, res


def _dilate(v):
    out = v.copy()
    out[:-1, :] |= v[1:, :]
    out[1:, :] |= v[:-1, :]
    out[:, :-1] |= v[:, 1:]
    out[:, 1:] |= v[:, :-1]
    return out


def _host_exact_maze_terms(noise, Wg):
    """Fallback (practically unreachable): exact wall/flood-fill computation."""
    solv = 0.0
    wall_total = 0
    for b0 in range(0, B, 64):
        x = noise[b0 : b0 + 64].astype(np.float32) @ Wg.astype(np.float32)
        fake = np.tanh(x).astype(np.float32)
        for j in range(fake.shape[0]):
            maze = fake[j].reshape(H, W)
            wall = maze == np.float32(1.0)
            nwall = int(wall.sum())
            wall_total += nwall
            pen = 0.0
            if float(wall.mean()) > 0.5:
                pen += 1.0
            if nwall >= 3:
                open_ = ~wall
                visited = np.zeros((H, W), bool)
                visited[1, 1] = True
                while True:
                    nv = visited | (_dilate(visited) & open_)
                    if not (nv & ~visited).any():
                        break
                    visited = nv
                wf = wall.astype(np.float32)
                wa = np.zeros((H, W), np.float32)
                wa[:-1, :] += wf[1:, :]
                wa[1:, :] += wf[:-1, :]
                wa[:, :-1] += wf[:, 1:]
                wa[:, 1:] += wf[:, :-1]
                pen += 0.1 * float((visited & (wa >= 3.0)).sum())
            solv += pen
    solv /= B
    cur = wall_total / float(B * H * W)
    return solv, cur


def kernel(**inputs) -> np.ndarray:
    noise = np.asarray(inputs["noise"], np.float32)
    Wg = np.asarray(inputs["Wg"], np.float32)
    Wd = np.asarray(inputs["Wd"], np.float32)
    p = float(np.asarray(inputs["maml_performance"]).reshape(-1)[0])
    cd = float(np.asarray(inputs["current_difficulty"]).reshape(-1)[0])

    d_fake, _ = run_device(noise, Wg, Wd)

    # g_loss = mean(softplus(-d_fake));  0.0 * sum(d_real) == 0 exactly.
    g_loss = float(np.mean(np.logaddexp(0.0, -d_fake)))

    # Wall existence bound: |x[b,n]| <= max_b||noise_b|| * max_n||Wg[:,n]||.
    rn = float(np.sqrt((noise.astype(np.float64) ** 2).sum(axis=1)).max())
    cn = float(np.sqrt((Wg.astype(np.float64) ** 2).sum(axis=0)).max())
    if rn * cn * 1.0001 < WALL_SAFE_BOUND:
        solv, cur = 0.0, 0.0
    else:  # pragma: no cover - requires |pre-tanh| ~ 28 sigma
        solv, cur = _host_exact_maze_terms(noise, Wg)

    w_s = 0.8 if p < 0.4 else (0.4 if p > 0.6 else 0.6)
    w_d = 0.05 if p < 0.4 else (0.2 if p > 0.6 else 0.1)
    difficulty = (cur - cd) ** 2
    loss = g_loss + w_s * solv + w_d * difficulty
    return np.array(loss, dtype=np.float32)
